# revision 31
# baseline (speedup 1.0000x reference)
"""GATr-style geometric-algebra transformer block on 8 Trainium2 NeuronCores.

v2 rework of the fp16x2/fp8 baseline:
- q/k projections: fp16 hi-term (W16.hh) + fp8 DoubleRow lo-term
  (W8lo.hl8, scale 512 folded into e5m2 weights / e4m3 residuals) -
  baseline-class accuracy at ~60% of the PE cost.
- scores: single fp16 product (qh.kh), fp32 accumulate (as baseline).
- attn@v / out-projection / v-projection: fp8 e4m3 DoubleRow.
- norm-reduce (ipc) and final projection run as float32r (FP22 on HW:
  plenty for a normalizer and the output path) - 4x cheaper than fp32.
- Phase A computes square/norm/residual directly from the enter-psum
  (no hT round-trip): shorter startup critical path.
- Out-projection accumulates 4-head groups in PSUM via a persistent oT
  buffer, so the residual add runs once per (group, qg) not per head.
- Sum-reciprocal transposed to the free axis in bf16 (1 cyc/row).
- Row-max: two wide DVE reduces per q-tile + tiny negated combine
  (DVE reads at most one PSUM input; gpsimd none - no cheaper scheme).

Sharding: 8 cores = 4 batches x 2 query-halves (token axis rotated per core
so queries are always tokens [0, 1024)); no cross-core communication.
"""

import sys

import numpy as np

for _p in ("/opt/trn_rl_repo", "/root/.axon_site/_ro/trn_rl_repo"):
    if _p not in sys.path:
        sys.path.insert(0, _p)

import ml_dtypes  # noqa: E402

import concourse.bacc as bacc  # noqa: E402
import concourse.tile as tile  # noqa: E402
from concourse import mybir  # noqa: E402
from concourse.bass_utils import run_bass_kernel_spmd  # noqa: E402

F32 = mybir.dt.float32
F32R = mybir.dt.float32r
F16 = mybir.dt.float16
BF16 = mybir.dt.bfloat16
F8 = mybir.dt.float8e4
F8_5 = mybir.dt.float8e5
DROW = mybir.MatmulPerfMode.DoubleRow
AX = mybir.AxisListType.X
AF = mybir.ActivationFunctionType

INNER = np.array([0, 2, 3, 4, 8, 9, 10, 14])
B, S, C_IN, MV = 4, 2048, 3, 16
HID, NH = 32, 8
NCORES = 8
SLAB = S // 2  # queries per core

TRACE = False
LAST_RESULTS = None


# --------------------------------------------------------------------------
# Device program
# --------------------------------------------------------------------------

def _emit(tc):
    nc = tc.nc

    xT_d = nc.declare_dram_parameter("xT", [48, S], F32, isOutput=False)
    w_enter_d = nc.declare_dram_parameter("w_enter", [48, 512], F32, isOutput=False)
    w_q_d = nc.declare_dram_parameter("w_q", [128, 4, 2048], F16, isOutput=False)
    w_q8_d = nc.declare_dram_parameter("w_q8", [128, 4, 2048], F8_5, isOutput=False)
    w_k_d = nc.declare_dram_parameter("w_k", [128, 4, 256], F16, isOutput=False)
    w_k8_d = nc.declare_dram_parameter("w_k8", [128, 4, 256], F8_5, isOutput=False)
    w_v_d = nc.declare_dram_parameter("w_v", [128, 4, 512], F8, isOutput=False)
    w_out_d = nc.declare_dram_parameter("w_out", [128, 32, 512], F8, isOutput=False)
    w_fin_d = nc.declare_dram_parameter("w_final", [128, 4, 512], F32R, isOutput=False)
    mask_d = nc.declare_dram_parameter("mask", [128, 4], F32R, isOutput=False)
    idb_d = nc.declare_dram_parameter("ident_b", [128, 128], BF16, isOutput=False)
    idh_d = nc.declare_dram_parameter("ident_h", [128, 128], F8, isOutput=False)
    outT_d = nc.declare_dram_parameter("outT", [512, SLAB], F32, isOutput=True)

    from contextlib import ExitStack

    with ExitStack() as ctx:
        psum = ctx.enter_context(tc.tile_pool(name="ps", bufs=1, space="PSUM"))
        pp = ctx.enter_context(tc.tile_pool(name="persist", bufs=1))

        # ---- persistent tiles -------------------------------------------
        hh = pp.tile([128, 4, S], F16, name="hh")
        hl8 = pp.tile([128, 4, S], F8_5, name="hl8")
        hh8 = pp.tile([128, 4, S], F8, name="hh8")
        kh = pp.tile([128, 2, S], F16, name="kh")
        v_tok = pp.tile([128, 16, 512], F8, name="v_tok")
        acc = pp.tile([128, 4, SLAB], F32R, name="acc")
        w_fin = pp.tile([128, 4, 512], F32R, name="w_fin")
        w_v = pp.tile([128, 4, 512], F8, name="w_v")
        w_out = pp.tile([128, 32, 512], F8, name="w_out")
        # oT collects normalized per-head attn outputs for BOTH query
        # groups of a 4-head group: [qg, head%4 * 4 + k-chunk, q]
        oT = pp.tile([128, 2, 16, 512], F8, name="oT")
        mask = pp.tile([128, 4], F32R, name="mask")
        idb = pp.tile([128, 128], BF16, name="idb")
        idh = pp.tile([128, 128], F8, name="idh")

        # ================= phase A: enter, norm, k/v =====================
        with tc.tile_pool(name="pA", bufs=1) as pA:
            xT = pA.tile([48, S], F32, name="xT")
            w_enter = pA.tile([48, 512], F32, name="w_enter")
            w_ksh = pA.tile([128, 4, 256], F16, name="w_ksh")
            w_ksh8 = pA.tile([128, 4, 256], F8_5, name="w_ksh8")
            hn = pA.tile([128, 4, S], F32, name="hn")
            sq_c = [pA.tile([1, 512], F32, tag="sqc", bufs=2, name="sq_c")
                    for _ in range(4)]
            rt_c = [pA.tile([1, 512], F32, tag="rtc", bufs=2, name="rt_c")
                    for _ in range(4)]
            rb_c = [pA.tile([128, 512], F32, tag="rbc", bufs=2, name="rb_c")
                    for _ in range(4)]
            # enter-critical transfers first (first matmul can start as
            # soon as chunk 0 + weights land); big weights follow by need
            nc.sync.dma_start(out=xT[:, 0:512], in_=xT_d[:, 0:512])
            # weights on the ACT DGE queue: dispatches in parallel with xT
            nc.scalar.dma_start(out=w_enter[:], in_=w_enter_d[:, :])
            # small early-needed tensors before the non-critical xT chunks
            nc.sync.dma_start(out=mask[:], in_=mask_d[:, :])
            nc.sync.dma_start(out=xT[:, 512:1024], in_=xT_d[:, 512:1024])
            nc.sync.dma_start(out=w_ksh[:], in_=w_k_d[:, :, :])
            nc.sync.dma_start(out=w_ksh8[:], in_=w_k8_d[:, :, :])
            for cc in range(2, 4):
                nc.sync.dma_start(out=xT[:, cc * 512:(cc + 1) * 512],
                                  in_=xT_d[:, cc * 512:(cc + 1) * 512])
            nc.sync.dma_start(out=idh[:], in_=idh_d[:, :])
            nc.sync.dma_start(out=idb[:], in_=idb_d[:, :])
            # big weights ride the SP queue so the ACT DGE queue stays
            # free for the per-head q-weight fetches
            nc.sync.dma_start(out=w_v[:], in_=w_v_d[:, :, :])
            nc.sync.dma_start(out=w_fin[:], in_=w_fin_d[:, :, :])
            nc.sync.dma_start(out=w_out[:], in_=w_out_d[:, :, :])

            hT = pA.tile([128, 4, S], F32, name="hT")

            def emit_enter(cc):
                # enter matmuls + masked-square reduce + early psum->SBUF
                # evacuation (splits ACT/DVE); psum frees after ~3.5us so
                # the 3-slot sc rotation sustains 2 chunks in flight
                cs = slice(cc * 512, (cc + 1) * 512)
                eps = [psum.tile([128, 2, 512], F32, tag="sc", bufs=3,
                                 name="eps") for _ in range(2)]
                for mt in range(4):
                    nc.tensor.matmul(
                        eps[mt // 2][:, mt % 2, :],
                        w_enter[0:48, mt * 128:(mt + 1) * 128],
                        xT[0:48, cs],
                        start=True, stop=True,
                    )
                ipc = psum.tile([1, 512], F32, tag="av", bufs=2, name="ipc")
                for mt in range(4):
                    hsq = pA.tile([128, 512], F32R, tag="hsq", bufs=3,
                                  name="hsq")
                    nc.scalar.activation(hsq[:], eps[mt // 2][:, mt % 2, :],
                                         AF.Square)
                    nc.tensor.matmul(
                        ipc[:],
                        mask[:, mt:mt + 1],
                        hsq[:],
                        start=(mt == 0), stop=(mt == 3),
                    )
                for mt in range(4):
                    if mt < 2:
                        nc.scalar.copy(hT[:, mt, cs],
                                       eps[mt // 2][:, mt % 2, :])
                    else:
                        nc.vector.tensor_copy(hT[:, mt, cs],
                                              eps[mt // 2][:, mt % 2, :])
                nc.scalar.activation(sq_c[cc][:], ipc[:], AF.Sqrt)

            def emit_norm(cc):
                cs = slice(cc * 512, (cc + 1) * 512)
                nc.vector.reciprocal(rt_c[cc][:], sq_c[cc][:])
                nc.gpsimd.partition_broadcast(rb_c[cc][:], rt_c[cc][0:1, :])
                for mt in range(4):
                    # hn = h / sqrt(mean sq); fp16 hi + e5m2 lo split
                    # (the residual fits e5m2's normal range unscaled)
                    nc.vector.tensor_mul(hn[:, mt, cs], hT[:, mt, cs],
                                         rb_c[cc][:])
                    nc.scalar.copy(hh[:, mt, cs], hn[:, mt, cs])
                    nc.vector.tensor_sub(hl8[:, mt, cs], hn[:, mt, cs],
                                         hh[:, mt, cs])

            def emit_ks(cc):
                # ks projection: fp16 hi + fp8 DoubleRow lo
                cs = slice(cc * 512, (cc + 1) * 512)
                for mt in range(2):
                    ps = psum.tile([128, 512], F32, tag="av", bufs=2,
                                   name="ps_ks")
                    for kt in range(4):
                        nc.tensor.matmul(
                            ps[:],
                            w_ksh[:, kt, mt * 128:(mt + 1) * 128],
                            hh[:, kt, cs],
                            start=(kt == 0), stop=False,
                        )
                    for s2 in range(2):
                        nc.tensor.matmul(
                            ps[:],
                            w_ksh8[:, 2 * s2:2 * s2 + 2,
                                   mt * 128:(mt + 1) * 128],
                            hl8[:, 2 * s2:2 * s2 + 2, cs],
                            start=False, stop=(s2 == 1),
                            perf_mode=DROW,
                        )
                    nc.vector.tensor_copy(kh[:, mt, cs], ps[:])

            def emit_late(cc):
                # off the ks critical path: residual + fp8 shadow of hh
                cs = slice(cc * 512, (cc + 1) * 512)
                for mt in range(4):
                    if cc < 2:
                        # residual lives in acc (queries = tokens [0, SLAB))
                        nc.gpsimd.tensor_copy(acc[:, mt, cs], hT[:, mt, cs])
                    nc.gpsimd.tensor_copy(hh8[:, mt, cs], hh[:, mt, cs])

            def emit_v(j):
                # v-projection piece for token chunk j (fp8 DoubleRow)
                for tt in range(4 * j, 4 * j + 4):
                    ps = psum.tile([128, 512], F32, tag="av", bufs=2,
                                   name="ps_v")
                    for k2 in range(2):
                        nc.tensor.matmul(
                            ps[:],
                            hh8[:, 2 * k2:2 * k2 + 2, tt * 128:(tt + 1) * 128],
                            w_v[:, 2 * k2:2 * k2 + 2, :],
                            start=(k2 == 0), stop=(k2 == 1),
                            perf_mode=DROW,
                        )
                    nc.vector.tensor_copy(v_tok[:, tt, :], ps[:])

            # software-pipelined emission: chunk cc+1's enter/ipc matmuls
            # reach the in-order PE queue before ks(cc)
            emit_enter(0)
            emit_enter(1)
            emit_norm(0)
            emit_enter(2)
            emit_ks(0)
            emit_norm(1)
            emit_enter(3)
            emit_ks(1)
            emit_late(0)
            emit_norm(2)
            emit_ks(2)
            emit_late(1)
            emit_norm(3)
            emit_ks(3)
            emit_late(2)
            emit_late(3)

        # ================= phase B: attention over 8 heads ===============
        with tc.tile_pool(name="pB", bufs=1) as pB:
            pending = [[(lambda jj: (lambda: emit_v(jj)))(j)
                        for j in range(4)]]

            def transposes(attn_q, attnT, qt):
                # qt-major: 16 transposed blocks of attn[qt] -> attnT.
                # fp8 transposes write with element step 2; move the
                # stride-2 blocks (holes included) as packed f32 words.
                for w in range(2):
                    tr = psum.tile([128, 2048], F8, tag="av", bufs=2, name="tr")
                    for k in range(8):
                        tt = w * 8 + k
                        nc.tensor.transpose(
                            tr[:, k * 256:(k + 1) * 256:2],
                            attn_q[qt][:, tt * 128:(tt + 1) * 128],
                            idh[:],
                        )
                    src = tr[:].bitcast(F32).rearrange("p (k q) -> p k q", k=8)
                    dst = attnT[:].bitcast(F32)[
                        :, w * 8:(w + 1) * 8, qt * 64:(qt + 1) * 64]
                    # ACT only: keeps DVE free for the softmax reduce chain
                    nc.scalar.copy(dst, src)

            def make_finisher(attn_q, attnT, sexp, h, qg, extras=None):
                # Four pieces, fired one per q-tile of the NEXT scores block,
                # so PE always has independent work while softmax chains run.
                state = {}
                hq = h % 4  # slot within the 4-head out-projection group

                def p0():
                    transposes(attn_q, attnT, 2)
                    if extras:
                        extras[0]()

                def sumrecip():
                    # reciprocal on the compact [128,4] sums, bf16 shadow,
                    # then transpose to the free axis and broadcast
                    rsx = pB.tile([128, 4], F32, tag="rsx", bufs=2, name="rsx")
                    nc.vector.reciprocal(rsx[:], sexp[:])
                    rsxh = pB.tile([128, 4], BF16, tag="rsxh", bufs=2,
                                   name="rsxh")
                    nc.vector.tensor_copy(rsxh[:], rsx[:])
                    st = psum.tile([1, 512], BF16, tag="av", bufs=2, name="st")
                    for qt in range(4):
                        nc.tensor.transpose(
                            st[0:1, qt * 128:(qt + 1) * 128],
                            rsxh[:, qt:qt + 1], idb[:],
                        )
                    rqT = pB.tile([1, 512], BF16, tag="rqT", bufs=2, name="rqT")
                    nc.scalar.copy(rqT[:], st[0:1, :])
                    rb_q = pB.tile([128, 512], BF16, tag="rbq", bufs=2,
                                   name="rb_q")
                    nc.gpsimd.partition_broadcast(rb_q[:], rqT[0:1, :])
                    state["rb_q"] = rb_q

                def av(mt):
                    avp = psum.tile([128, 512], F32, tag="av", bufs=2, name="avp")
                    for t2 in range(8):
                        nc.tensor.matmul(
                            avp[:],
                            v_tok[:, 2 * t2:2 * t2 + 2, mt * 128:(mt + 1) * 128],
                            attnT[:, 2 * t2:2 * t2 + 2, 0:1024:2],
                            start=(t2 == 0), stop=(t2 == 7),
                            perf_mode=DROW,
                        )
                    nc.vector.tensor_mul(oT[:, qg, hq * 4 + mt, :], avp[:],
                                         state["rb_q"])

                def p1():
                    transposes(attn_q, attnT, 3)
                    sumrecip()
                    if extras:
                        extras[1]()

                def p2():
                    av(0)
                    av(1)
                    av(2)
                    av(3)
                    if extras:
                        extras[2]()

                def p3():
                    if hq == 3:
                        # out-projection for the whole 4-head group, this qg:
                        # fp8 DoubleRow, 8 accumulation steps per mt tile
                        g = h // 4
                        for mt2 in range(4):
                            opp = psum.tile([128, 512], F32, tag="av", bufs=2,
                                            name="opp")
                            for k8 in range(8):
                                nc.tensor.matmul(
                                    opp[:],
                                    w_out[:, 16 * g + 2 * k8:16 * g + 2 * k8 + 2,
                                          mt2 * 128:(mt2 + 1) * 128],
                                    oT[:, qg, 2 * k8:2 * k8 + 2, :],
                                    start=(k8 == 0), stop=(k8 == 7),
                                    perf_mode=DROW,
                                )
                            sl = acc[:, mt2, qg * 512:(qg + 1) * 512]
                            nc.vector.tensor_add(sl, sl, opp[:])
                    if extras:
                        extras[3]()
                return [p0, p1, p2, p3]

            def fetch_head(h):
                # issue the head's q-weight transfers (one head ahead)
                wq = pB.tile([128, 4, 256], F16, tag="wq", bufs=2, name="w_q")
                nc.scalar.dma_start(out=wq[:],
                                    in_=w_q_d[:, :, h * 256:(h + 1) * 256])
                wq8 = pB.tile([128, 4, 256], F8_5, tag="wq8", bufs=2,
                              name="w_q8")
                nc.scalar.dma_start(out=wq8[:],
                                    in_=w_q8_d[:, :, h * 256:(h + 1) * 256])
                return wq, wq8

            def qs_group(wq, wq8, qh_t, mt, cc):
                # one qs-projection group: fp16 hi + fp8 DoubleRow lo
                def go():
                    ps = psum.tile([128, 512], F32, tag="av", bufs=2,
                                   name="ps_qs")
                    for kt in range(4):
                        nc.tensor.matmul(
                            ps[:],
                            wq[:, kt, mt * 128:(mt + 1) * 128],
                            hh[:, kt, cc * 512:(cc + 1) * 512],
                            start=(kt == 0), stop=False,
                        )
                    for s2 in range(2):
                        nc.tensor.matmul(
                            ps[:],
                            wq8[:, 2 * s2:2 * s2 + 2, mt * 128:(mt + 1) * 128],
                            hl8[:, 2 * s2:2 * s2 + 2,
                                cc * 512:(cc + 1) * 512],
                            start=False, stop=(s2 == 1),
                            perf_mode=DROW,
                        )
                    cs = slice(cc * 512, (cc + 1) * 512)
                    # ACT evac: keeps DVE's in-order queue free for the
                    # row-max chain that paces each block
                    nc.scalar.copy(qh_t[:, mt, cs], ps[:])
                return go

            # tail: interleave final-projection chunks with the last
            # finisher's pieces (w_fin already resident from phase A)
            def final_chunk(cc):
                for mt in range(4):
                    fp = psum.tile([128, 512], F32, tag="av", bufs=2, name="fp")
                    for kt in range(4):
                        nc.tensor.matmul(
                            fp[:],
                            w_fin[:, kt, mt * 128:(mt + 1) * 128],
                            acc[:, kt, cc * 512:(cc + 1) * 512],
                            start=(kt == 0), stop=(kt == 3),
                        )
                    outc = pB.tile([128, 512], F32, tag="outc", bufs=2,
                                   name="outc")
                    nc.scalar.copy(outc[:], fp[:])
                    nc.sync.dma_start(
                        out=outT_d[mt * 128:(mt + 1) * 128,
                                   cc * 512:(cc + 1) * 512],
                        in_=outc[:],
                    )

            def _noop():
                pass

            tail_extras = [_noop, _noop, _noop, lambda: final_chunk(0)]

            nxt_w = fetch_head(0)
            # head 0's qs projection runs inline; later heads are emitted as
            # extra finisher pieces during the previous head's second half
            qh = pB.tile([128, 2, SLAB], F16, tag="qh", bufs=2, name="qh")
            for mt in range(2):
                for cc in range(2):
                    qs_group(nxt_w[0], nxt_w[1], qh, mt, cc)()

            for h in range(NH):
                w_qh, w_q8h = nxt_w
                if h + 1 < NH:
                    nxt_w = fetch_head(h + 1)
                    qh_next = pB.tile([128, 2, SLAB], F16, tag="qh", bufs=2,
                                      name="qh")
                    qs_extras = [qs_group(nxt_w[0], nxt_w[1], qh_next, mt, cc)
                                 for mt in range(2) for cc in range(2)]
                else:
                    qh_next, qs_extras = None, None

                for qg in range(SLAB // 512):
                    attn_q = []
                    sexp = pB.tile([128, 4], F32, tag="sexp", bufs=2, name="sexp")
                    attnT = pB.tile([128, 16, 1024], F8, tag="attnT",
                                    bufs=2, name="attnT")

                    for qt in range(4):
                        qtg = qg * 4 + qt
                        qsl = slice(qtg * 128, (qtg + 1) * 128)
                        # two 2-bank PSUM tiles; each pair of score chunks
                        # max-reduces in a single [128, 2x512] DVE pass
                        sc_p = [psum.tile([128, 2, 512], F32, tag="sc", bufs=3,
                                          name="scp") for _ in range(2)]
                        sc_c = [sc_p[t4 // 2][:, t4 % 2, :] for t4 in range(4)]
                        # scores = qh*kh  (single fp16 product, fp32 acc);
                        # pair-major so each pair's row-max can start while
                        # the later chunks are still multiplying
                        for t4 in range(4):
                            for si in range(2):
                                nc.tensor.matmul(
                                    sc_c[t4],
                                    qh[:, si, qsl],
                                    kh[:, si, t4 * 512:(t4 + 1) * 512],
                                    start=(si == 0), stop=(si == 1),
                                )
                        negmax = pB.tile([128, 1], F32, tag="negmax", bufs=4,
                                         name="negmax")
                        cmax = pB.tile([128, 2], F32, tag="cmax", bufs=4,
                                       name="cmax")
                        for pr in range(2):
                            nc.vector.reduce_max(cmax[:, pr:pr + 1],
                                                 sc_p[pr][:],
                                                 axis=mybir.AxisListType.XY)
                        nc.vector.reduce_max(negmax[:], cmax[:], axis=AX,
                                             negate=True)
                        attn = pB.tile([128, S], F8, tag="attn", bufs=5, name="attn")
                        seh = pB.tile([128, 2], F32, tag="seh", bufs=4, name="seh")
                        for pr in range(2):
                            nc.scalar.activation(
                                attn[:, pr * 1024:(pr + 1) * 1024].rearrange(
                                    "p (a b) -> p a b", a=2),
                                sc_p[pr][:], AF.Exp,
                                bias=negmax[:], scale=1.0,
                                accum_out=seh[:, pr:pr + 1],
                            )
                        nc.vector.reduce_sum(sexp[:, qt:qt + 1], seh[:], axis=AX)
                        attn_q.append(attn)
                        if pending[0] is not None:
                            pending[0][qt]()
                        if qt >= 2:
                            transposes(attn_q, attnT, qt - 2)

                    if qg == 0:
                        fin_extras = qs_extras if qs_extras else tail_extras
                    else:
                        fin_extras = None
                    pending[0] = make_finisher(
                        attn_q, attnT, sexp, h, qg, extras=fin_extras)

                qh = qh_next

            # tail
            ps_ = pending[0]
            ps_[0]()
            ps_[1]()
            ps_[2]()
            ps_[3]()
            final_chunk(1)


_NC_CACHE = None


def _get_nc():
    global _NC_CACHE
    if _NC_CACHE is None:
        nc = bacc.Bacc("TRN2", debug=False, num_devices=NCORES)
        with tile.TileContext(nc) as tc:
            _emit(tc)
        nc.compile()
        _NC_CACHE = nc
    return _NC_CACHE


# --------------------------------------------------------------------------
# Host-side packing
# --------------------------------------------------------------------------

def _fuse(w, blade):
    # einsum('jib,bxy->jixy', w, blade) -> mat[(i,x), (j,y)]
    wb = np.einsum("jib,bxy->jixy", w, blade)
    j, i = w.shape[0], w.shape[1]
    return np.ascontiguousarray(wb.transpose(1, 2, 0, 3)).reshape(i * 16, j * 16)


def _to_kt(m, kparts):
    # [K, F] -> [128, K//128, F] partition-major packing
    k, f = m.shape
    assert k == kparts * 128
    return np.ascontiguousarray(m.reshape(kparts, 128, f).transpose(1, 0, 2))


def _pack_weights(blade, w_enter, w_q, w_k, w_v, w_out, w_final):
    f32 = np.float32
    W_enter = _fuse(w_enter, blade).astype(f32)  # [48, 512]

    Wq = _fuse(w_q, blade)  # [512, 4096], col (j, y), j = d*8 + h
    Wq = Wq.reshape(512, 32, 8, 16)[:, :, :, INNER]   # [c, d, h, yi]
    Wq = Wq.transpose(0, 2, 1, 3).reshape(512, 8, 256)  # [c, h, (d,yi)]
    Wqs = (Wq.reshape(512, 2048) / 16.0).astype(f32)   # fold 1/sqrt(256)

    Wk = _fuse(w_k, blade)  # [512, 512], col (d, y)
    Wks = Wk.reshape(512, 32, 16)[:, :, INNER].reshape(512, 256).astype(f32)

    Wv = _fuse(w_v, blade).astype(f32)       # [512, 512]
    Wo = _fuse(w_out, blade).astype(f32)     # [4096, 512], rows (h, d, x)
    Wf = _fuse(w_final, blade).astype(f32)   # [512, 512]

    maskv = np.zeros(512, f32)
    for d in range(32):
        maskv[d * 16 + INNER] = 1.0 / 32.0
    mask = np.ascontiguousarray(maskv.reshape(4, 128).T)

    return {
        "w_enter": W_enter,
        "w_q": _to_kt(Wqs, 4).astype(np.float16),
        "w_q8": _to_kt(Wqs, 4).astype(ml_dtypes.float8_e5m2),
        "w_k": _to_kt(Wks, 4).astype(np.float16),
        "w_k8": _to_kt(Wks, 4).astype(ml_dtypes.float8_e5m2),
        "w_v": _to_kt(Wv, 4).astype(ml_dtypes.float8_e4m3),
        "w_out": _to_kt(Wo, 32).astype(ml_dtypes.float8_e4m3),
        "w_final": _to_kt(Wf, 4),
        "mask": mask,
        "ident_b": np.eye(128).astype(ml_dtypes.bfloat16),
        "ident_h": np.eye(128).astype(ml_dtypes.float8_e4m3),
    }


def kernel(x, blade, w_enter, w_q, w_k, w_v, w_out, w_final):
    global LAST_RESULTS
    x = np.asarray(x, np.float32)
    shared = _pack_weights(
        np.asarray(blade, np.float32), np.asarray(w_enter, np.float32),
        np.asarray(w_q, np.float32), np.asarray(w_k, np.float32),
        np.asarray(w_v, np.float32), np.asarray(w_out, np.float32),
        np.asarray(w_final, np.float32),
    )

    in_maps = []
    for c in range(NCORES):
        b, half = c // 2, c % 2
        xb = x[b].reshape(S, 48)
        xb = np.roll(xb, -SLAB * half, axis=0)
        m = dict(shared)
        m["xT"] = np.ascontiguousarray(xb.T)
        in_maps.append(m)

    nc = _get_nc()
    res = run_bass_kernel_spmd(
        nc, in_maps, core_ids=list(range(NCORES)), trace=TRACE,
    )
    LAST_RESULTS = res

    out = np.empty((B, S, HID, MV), np.float32)
    for c in range(NCORES):
        b, half = c // 2, c % 2
        outT = res.results[c]["outT"]  # [512, 1024]
        out[b, half * SLAB:(half + 1) * SLAB] = (
            outT.T.reshape(SLAB, HID, MV)
        )
    return out


# revision 33
# speedup vs baseline: 1.0245x; 1.0245x over previous
"""GATr-style geometric-algebra transformer block on 8 Trainium2 NeuronCores.

v2 rework of the fp16x2/fp8 baseline:
- q/k projections: fp16 hi-term (W16.hh) + fp8 DoubleRow lo-term
  (W8lo.hl8, scale 512 folded into e5m2 weights / e4m3 residuals) -
  baseline-class accuracy at ~60% of the PE cost.
- scores: single fp16 product (qh.kh), fp32 accumulate (as baseline).
- attn@v / out-projection / v-projection: fp8 e4m3 DoubleRow.
- norm-reduce (ipc) and final projection run as float32r (FP22 on HW:
  plenty for a normalizer and the output path) - 4x cheaper than fp32.
- Phase A computes square/norm/residual directly from the enter-psum
  (no hT round-trip): shorter startup critical path.
- Out-projection accumulates 4-head groups in PSUM via a persistent oT
  buffer, so the residual add runs once per (group, qg) not per head.
- Sum-reciprocal transposed to the free axis in bf16 (1 cyc/row).
- Row-max: two wide DVE reduces per q-tile + tiny negated combine
  (DVE reads at most one PSUM input; gpsimd none - no cheaper scheme).

Sharding: 8 cores = 4 batches x 2 query-halves (token axis rotated per core
so queries are always tokens [0, 1024)); no cross-core communication.
"""

import sys

import numpy as np

for _p in ("/opt/trn_rl_repo", "/root/.axon_site/_ro/trn_rl_repo"):
    if _p not in sys.path:
        sys.path.insert(0, _p)

import ml_dtypes  # noqa: E402

import concourse.bacc as bacc  # noqa: E402
import concourse.tile as tile  # noqa: E402
from concourse import mybir  # noqa: E402
from concourse.bass_utils import run_bass_kernel_spmd  # noqa: E402

F32 = mybir.dt.float32
F32R = mybir.dt.float32r
F16 = mybir.dt.float16
BF16 = mybir.dt.bfloat16
F8 = mybir.dt.float8e4
F8_5 = mybir.dt.float8e5
DROW = mybir.MatmulPerfMode.DoubleRow
AX = mybir.AxisListType.X
AF = mybir.ActivationFunctionType

INNER = np.array([0, 2, 3, 4, 8, 9, 10, 14])
B, S, C_IN, MV = 4, 2048, 3, 16
HID, NH = 32, 8
NCORES = 8
SLAB = S // 2  # queries per core

TRACE = False
LAST_RESULTS = None


# --------------------------------------------------------------------------
# Device program
# --------------------------------------------------------------------------

def _emit(tc):
    nc = tc.nc

    xT_d = nc.declare_dram_parameter("xT", [48, S], F32, isOutput=False)
    w_enter_d = nc.declare_dram_parameter("w_enter", [48, 512], F32, isOutput=False)
    w_q_d = nc.declare_dram_parameter("w_q", [128, 4, 2048], F16, isOutput=False)
    w_q8_d = nc.declare_dram_parameter("w_q8", [128, 4, 2048], F8_5, isOutput=False)
    w_k_d = nc.declare_dram_parameter("w_k", [128, 4, 256], F16, isOutput=False)
    w_k8_d = nc.declare_dram_parameter("w_k8", [128, 4, 256], F8_5, isOutput=False)
    w_v_d = nc.declare_dram_parameter("w_v", [128, 4, 512], F8, isOutput=False)
    w_out_d = nc.declare_dram_parameter("w_out", [128, 32, 512], F8, isOutput=False)
    w_fin_d = nc.declare_dram_parameter("w_final", [128, 4, 512], F32R, isOutput=False)
    mask_d = nc.declare_dram_parameter("mask", [128, 4], F32R, isOutput=False)
    idb_d = nc.declare_dram_parameter("ident_b", [128, 128], BF16, isOutput=False)
    idh_d = nc.declare_dram_parameter("ident_h", [128, 128], F8, isOutput=False)
    outT_d = nc.declare_dram_parameter("outT", [512, SLAB], F32, isOutput=True)

    from contextlib import ExitStack

    with ExitStack() as ctx:
        psum = ctx.enter_context(tc.tile_pool(name="ps", bufs=1, space="PSUM"))
        pp = ctx.enter_context(tc.tile_pool(name="persist", bufs=1))

        # ---- persistent tiles -------------------------------------------
        hh = pp.tile([128, 4, S], F16, name="hh")
        hl8 = pp.tile([128, 4, S], F8_5, name="hl8")
        hh8 = pp.tile([128, 4, S], F8, name="hh8")
        kh = pp.tile([128, 2, S], F16, name="kh")
        v_tok = pp.tile([128, 16, 512], F8, name="v_tok")
        acc = pp.tile([128, 4, SLAB], F32R, name="acc")
        w_fin = pp.tile([128, 4, 512], F32R, name="w_fin")
        w_v = pp.tile([128, 4, 512], F8, name="w_v")
        w_out = pp.tile([128, 32, 512], F8, name="w_out")
        # oT collects normalized per-head attn outputs for BOTH query
        # groups of a 4-head group: [qg, head%4 * 4 + k-chunk, q]
        oT = pp.tile([128, 2, 16, 512], F8, name="oT")
        mask = pp.tile([128, 4], F32R, name="mask")
        idb = pp.tile([128, 128], BF16, name="idb")
        idh = pp.tile([128, 128], F8, name="idh")

        # ================= phase A: enter, norm, k/v =====================
        with tc.tile_pool(name="pA", bufs=1) as pA:
            xT = pA.tile([48, S], F32, name="xT")
            w_enter = pA.tile([48, 512], F32, name="w_enter")
            w_ksh = pA.tile([128, 4, 256], F16, name="w_ksh")
            w_ksh8 = pA.tile([128, 4, 256], F8_5, name="w_ksh8")
            hn = pA.tile([128, 4, S], F32, name="hn")
            sq_c = [pA.tile([1, 512], F32, tag="sqc", bufs=2, name="sq_c")
                    for _ in range(4)]
            rt_c = [pA.tile([1, 512], F32, tag="rtc", bufs=2, name="rt_c")
                    for _ in range(4)]
            rb_c = [pA.tile([128, 512], F32, tag="rbc", bufs=2, name="rb_c")
                    for _ in range(4)]
            # enter-critical transfers first (first matmul can start as
            # soon as chunk 0 + weights land); big weights follow by need
            nc.sync.dma_start(out=xT[:, 0:512], in_=xT_d[:, 0:512])
            # weights on the ACT DGE queue: dispatches in parallel with xT
            nc.scalar.dma_start(out=w_enter[:], in_=w_enter_d[:, :])
            # small early-needed tensors before the non-critical xT chunks
            nc.sync.dma_start(out=mask[:], in_=mask_d[:, :])
            nc.sync.dma_start(out=xT[:, 512:1024], in_=xT_d[:, 512:1024])
            nc.sync.dma_start(out=w_ksh[:], in_=w_k_d[:, :, :])
            nc.sync.dma_start(out=w_ksh8[:], in_=w_k8_d[:, :, :])
            for cc in range(2, 4):
                nc.sync.dma_start(out=xT[:, cc * 512:(cc + 1) * 512],
                                  in_=xT_d[:, cc * 512:(cc + 1) * 512])
            nc.sync.dma_start(out=idh[:], in_=idh_d[:, :])
            nc.sync.dma_start(out=idb[:], in_=idb_d[:, :])
            # big weights ride the SP queue so the ACT DGE queue stays
            # free for the per-head q-weight fetches
            nc.sync.dma_start(out=w_v[:], in_=w_v_d[:, :, :])
            nc.sync.dma_start(out=w_fin[:], in_=w_fin_d[:, :, :])
            nc.sync.dma_start(out=w_out[:], in_=w_out_d[:, :, :])

            hT = pA.tile([128, 4, S], F32, name="hT")

            def emit_enter(cc):
                # enter matmuls + masked-square reduce + early psum->SBUF
                # evacuation (splits ACT/DVE); psum frees after ~3.5us so
                # the 3-slot sc rotation sustains 2 chunks in flight
                cs = slice(cc * 512, (cc + 1) * 512)
                eps = [psum.tile([128, 2, 512], F32, tag="sc", bufs=3,
                                 name="eps") for _ in range(2)]
                for mt in range(4):
                    nc.tensor.matmul(
                        eps[mt // 2][:, mt % 2, :],
                        w_enter[0:48, mt * 128:(mt + 1) * 128],
                        xT[0:48, cs],
                        start=True, stop=True,
                    )
                ipc = psum.tile([1, 512], F32, tag="av", bufs=2, name="ipc")
                for mt in range(4):
                    hsq = pA.tile([128, 512], F32R, tag="hsq", bufs=3,
                                  name="hsq")
                    nc.scalar.activation(hsq[:], eps[mt // 2][:, mt % 2, :],
                                         AF.Square)
                    nc.tensor.matmul(
                        ipc[:],
                        mask[:, mt:mt + 1],
                        hsq[:],
                        start=(mt == 0), stop=(mt == 3),
                    )
                for mt in range(4):
                    if mt < 2:
                        nc.scalar.copy(hT[:, mt, cs],
                                       eps[mt // 2][:, mt % 2, :])
                    else:
                        nc.vector.tensor_copy(hT[:, mt, cs],
                                              eps[mt // 2][:, mt % 2, :])
                nc.scalar.activation(sq_c[cc][:], ipc[:], AF.Sqrt)

            def emit_norm(cc):
                cs = slice(cc * 512, (cc + 1) * 512)
                nc.vector.reciprocal(rt_c[cc][:], sq_c[cc][:])
                nc.gpsimd.partition_broadcast(rb_c[cc][:], rt_c[cc][0:1, :])
                for mt in range(4):
                    # hn = h / sqrt(mean sq); fp16 hi + e5m2 lo split
                    # (the residual fits e5m2's normal range unscaled)
                    nc.vector.tensor_mul(hn[:, mt, cs], hT[:, mt, cs],
                                         rb_c[cc][:])
                    nc.scalar.copy(hh[:, mt, cs], hn[:, mt, cs])
                    nc.vector.tensor_sub(hl8[:, mt, cs], hn[:, mt, cs],
                                         hh[:, mt, cs])

            def emit_ks(cc):
                # ks projection: fp16 hi + fp8 DoubleRow lo
                cs = slice(cc * 512, (cc + 1) * 512)
                for mt in range(2):
                    ps = psum.tile([128, 512], F32, tag="av", bufs=2,
                                   name="ps_ks")
                    for kt in range(4):
                        nc.tensor.matmul(
                            ps[:],
                            w_ksh[:, kt, mt * 128:(mt + 1) * 128],
                            hh[:, kt, cs],
                            start=(kt == 0), stop=False,
                        )
                    for s2 in range(2):
                        nc.tensor.matmul(
                            ps[:],
                            w_ksh8[:, 2 * s2:2 * s2 + 2,
                                   mt * 128:(mt + 1) * 128],
                            hl8[:, 2 * s2:2 * s2 + 2, cs],
                            start=False, stop=(s2 == 1),
                            perf_mode=DROW,
                        )
                    nc.vector.tensor_copy(kh[:, mt, cs], ps[:])

            def emit_late(cc):
                # off the ks critical path: residual + fp8 shadow of hh
                cs = slice(cc * 512, (cc + 1) * 512)
                for mt in range(4):
                    if cc < 2:
                        # residual lives in acc (queries = tokens [0, SLAB))
                        nc.gpsimd.tensor_copy(acc[:, mt, cs], hT[:, mt, cs])
                    nc.gpsimd.tensor_copy(hh8[:, mt, cs], hh[:, mt, cs])

            def emit_v(j):
                # v-projection piece for token chunk j (fp8 DoubleRow)
                for tt in range(4 * j, 4 * j + 4):
                    ps = psum.tile([128, 512], F32, tag="av", bufs=2,
                                   name="ps_v")
                    for k2 in range(2):
                        nc.tensor.matmul(
                            ps[:],
                            hh8[:, 2 * k2:2 * k2 + 2, tt * 128:(tt + 1) * 128],
                            w_v[:, 2 * k2:2 * k2 + 2, :],
                            start=(k2 == 0), stop=(k2 == 1),
                            perf_mode=DROW,
                        )
                    nc.vector.tensor_copy(v_tok[:, tt, :], ps[:])

            # software-pipelined emission: chunk cc+1's enter/ipc matmuls
            # reach the in-order PE queue before ks(cc)
            emit_enter(0)
            emit_enter(1)
            emit_norm(0)
            emit_enter(2)
            emit_norm(1)
            emit_enter(3)
            emit_ks(0)
            emit_ks(1)
            emit_norm(2)
            emit_ks(2)
            emit_late(0)
            emit_norm(3)
            emit_ks(3)
            emit_late(1)
            emit_late(2)
            emit_late(3)

        # ================= phase B: attention over 8 heads ===============
        with tc.tile_pool(name="pB", bufs=1) as pB:
            pending = [[(lambda jj: (lambda: emit_v(jj)))(j)
                        for j in range(4)]]

            def transposes(attn_q, attnT, qt):
                # qt-major: 16 transposed blocks of attn[qt] -> attnT.
                # fp8 transposes write with element step 2; move the
                # stride-2 blocks (holes included) as packed f32 words.
                for w in range(2):
                    tr = psum.tile([128, 2048], F8, tag="av", bufs=2, name="tr")
                    for k in range(8):
                        tt = w * 8 + k
                        nc.tensor.transpose(
                            tr[:, k * 256:(k + 1) * 256:2],
                            attn_q[qt][:, tt * 128:(tt + 1) * 128],
                            idh[:],
                        )
                    src = tr[:].bitcast(F32).rearrange("p (k q) -> p k q", k=8)
                    dst = attnT[:].bitcast(F32)[
                        :, w * 8:(w + 1) * 8, qt * 64:(qt + 1) * 64]
                    # ACT only: keeps DVE free for the softmax reduce chain
                    nc.scalar.copy(dst, src)

            def make_finisher(attn_q, attnT, sexp, h, qg, extras=None):
                # Four pieces, fired one per q-tile of the NEXT scores block,
                # so PE always has independent work while softmax chains run.
                state = {}
                hq = h % 4  # slot within the 4-head out-projection group

                def p0():
                    transposes(attn_q, attnT, 2)
                    if extras:
                        extras[0]()

                def sumrecip():
                    # reciprocal on the compact [128,4] sums, bf16 shadow,
                    # then transpose to the free axis and broadcast
                    rsx = pB.tile([128, 4], F32, tag="rsx", bufs=2, name="rsx")
                    nc.vector.reciprocal(rsx[:], sexp[:])
                    rsxh = pB.tile([128, 4], BF16, tag="rsxh", bufs=2,
                                   name="rsxh")
                    nc.vector.tensor_copy(rsxh[:], rsx[:])
                    st = psum.tile([1, 512], BF16, tag="av", bufs=2, name="st")
                    for qt in range(4):
                        nc.tensor.transpose(
                            st[0:1, qt * 128:(qt + 1) * 128],
                            rsxh[:, qt:qt + 1], idb[:],
                        )
                    rqT = pB.tile([1, 512], BF16, tag="rqT", bufs=2, name="rqT")
                    nc.scalar.copy(rqT[:], st[0:1, :])
                    rb_q = pB.tile([128, 512], BF16, tag="rbq", bufs=2,
                                   name="rb_q")
                    nc.gpsimd.partition_broadcast(rb_q[:], rqT[0:1, :])
                    state["rb_q"] = rb_q

                def av(mt):
                    avp = psum.tile([128, 512], F32, tag="av", bufs=2, name="avp")
                    for t2 in range(8):
                        nc.tensor.matmul(
                            avp[:],
                            v_tok[:, 2 * t2:2 * t2 + 2, mt * 128:(mt + 1) * 128],
                            attnT[:, 2 * t2:2 * t2 + 2, 0:1024:2],
                            start=(t2 == 0), stop=(t2 == 7),
                            perf_mode=DROW,
                        )
                    nc.vector.tensor_mul(oT[:, qg, hq * 4 + mt, :], avp[:],
                                         state["rb_q"])

                def p1():
                    transposes(attn_q, attnT, 3)
                    sumrecip()
                    if extras:
                        extras[1]()

                def p2():
                    av(0)
                    av(1)
                    av(2)
                    av(3)
                    if extras:
                        extras[2]()

                def p3():
                    if hq == 3:
                        # out-projection for the whole 4-head group, this qg:
                        # fp8 DoubleRow, 8 accumulation steps per mt tile
                        g = h // 4
                        for mt2 in range(4):
                            opp = psum.tile([128, 512], F32, tag="av", bufs=2,
                                            name="opp")
                            for k8 in range(8):
                                nc.tensor.matmul(
                                    opp[:],
                                    w_out[:, 16 * g + 2 * k8:16 * g + 2 * k8 + 2,
                                          mt2 * 128:(mt2 + 1) * 128],
                                    oT[:, qg, 2 * k8:2 * k8 + 2, :],
                                    start=(k8 == 0), stop=(k8 == 7),
                                    perf_mode=DROW,
                                )
                            sl = acc[:, mt2, qg * 512:(qg + 1) * 512]
                            nc.vector.tensor_add(sl, sl, opp[:])
                    if extras:
                        extras[3]()
                return [p0, p1, p2, p3]

            def fetch_head(h):
                # issue the head's q-weight transfers (one head ahead)
                wq = pB.tile([128, 4, 256], F16, tag="wq", bufs=2, name="w_q")
                nc.scalar.dma_start(out=wq[:],
                                    in_=w_q_d[:, :, h * 256:(h + 1) * 256])
                wq8 = pB.tile([128, 4, 256], F8_5, tag="wq8", bufs=2,
                              name="w_q8")
                nc.scalar.dma_start(out=wq8[:],
                                    in_=w_q8_d[:, :, h * 256:(h + 1) * 256])
                return wq, wq8

            def qs_group(wq, wq8, qh_t, mt, cc):
                # one qs-projection group: fp16 hi + fp8 DoubleRow lo
                def go():
                    ps = psum.tile([128, 512], F32, tag="av", bufs=2,
                                   name="ps_qs")
                    for kt in range(4):
                        nc.tensor.matmul(
                            ps[:],
                            wq[:, kt, mt * 128:(mt + 1) * 128],
                            hh[:, kt, cc * 512:(cc + 1) * 512],
                            start=(kt == 0), stop=False,
                        )
                    for s2 in range(2):
                        nc.tensor.matmul(
                            ps[:],
                            wq8[:, 2 * s2:2 * s2 + 2, mt * 128:(mt + 1) * 128],
                            hl8[:, 2 * s2:2 * s2 + 2,
                                cc * 512:(cc + 1) * 512],
                            start=False, stop=(s2 == 1),
                            perf_mode=DROW,
                        )
                    cs = slice(cc * 512, (cc + 1) * 512)
                    nc.vector.tensor_copy(qh_t[:, mt, cs], ps[:])
                return go

            # tail: interleave final-projection chunks with the last
            # finisher's pieces (w_fin already resident from phase A)
            def final_chunk(cc):
                for mt in range(4):
                    fp = psum.tile([128, 512], F32, tag="av", bufs=2, name="fp")
                    for kt in range(4):
                        nc.tensor.matmul(
                            fp[:],
                            w_fin[:, kt, mt * 128:(mt + 1) * 128],
                            acc[:, kt, cc * 512:(cc + 1) * 512],
                            start=(kt == 0), stop=(kt == 3),
                        )
                    outc = pB.tile([128, 512], F32, tag="outc", bufs=2,
                                   name="outc")
                    nc.scalar.copy(outc[:], fp[:])
                    nc.sync.dma_start(
                        out=outT_d[mt * 128:(mt + 1) * 128,
                                   cc * 512:(cc + 1) * 512],
                        in_=outc[:],
                    )

            def _noop():
                pass

            tail_extras = [_noop, _noop, _noop, lambda: final_chunk(0)]

            nxt_w = fetch_head(0)
            # head 0's qs projection runs inline; later heads are emitted as
            # extra finisher pieces during the previous head's second half
            qh = pB.tile([128, 2, SLAB], F16, tag="qh", bufs=2, name="qh")
            for mt in range(2):
                for cc in range(2):
                    qs_group(nxt_w[0], nxt_w[1], qh, mt, cc)()

            for h in range(NH):
                w_qh, w_q8h = nxt_w
                if h + 1 < NH:
                    nxt_w = fetch_head(h + 1)
                    qh_next = pB.tile([128, 2, SLAB], F16, tag="qh", bufs=2,
                                      name="qh")
                    qs_extras = [qs_group(nxt_w[0], nxt_w[1], qh_next, mt, cc)
                                 for mt in range(2) for cc in range(2)]
                else:
                    qh_next, qs_extras = None, None

                for qg in range(SLAB // 512):
                    attn_q = []
                    sexp = pB.tile([128, 4], F32, tag="sexp", bufs=2, name="sexp")
                    attnT = pB.tile([128, 16, 1024], F8, tag="attnT",
                                    bufs=2, name="attnT")

                    for qt in range(4):
                        qtg = qg * 4 + qt
                        qsl = slice(qtg * 128, (qtg + 1) * 128)
                        # two 2-bank PSUM tiles; each pair of score chunks
                        # max-reduces in a single [128, 2x512] DVE pass
                        sc_p = [psum.tile([128, 2, 512], F32, tag="sc", bufs=3,
                                          name="scp") for _ in range(2)]
                        sc_c = [sc_p[t4 // 2][:, t4 % 2, :] for t4 in range(4)]
                        # scores = qh*kh  (single fp16 product, fp32 acc);
                        # pair-major so each pair's row-max can start while
                        # the later chunks are still multiplying
                        for t4 in range(4):
                            for si in range(2):
                                nc.tensor.matmul(
                                    sc_c[t4],
                                    qh[:, si, qsl],
                                    kh[:, si, t4 * 512:(t4 + 1) * 512],
                                    start=(si == 0), stop=(si == 1),
                                )
                        negmax = pB.tile([128, 1], F32, tag="negmax", bufs=4,
                                         name="negmax")
                        cmax = pB.tile([128, 2], F32, tag="cmax", bufs=4,
                                       name="cmax")
                        for pr in range(2):
                            nc.vector.reduce_max(cmax[:, pr:pr + 1],
                                                 sc_p[pr][:],
                                                 axis=mybir.AxisListType.XY)
                        nc.vector.reduce_max(negmax[:], cmax[:], axis=AX,
                                             negate=True)
                        attn = pB.tile([128, S], F8, tag="attn", bufs=5, name="attn")
                        seh = pB.tile([128, 2], F32, tag="seh", bufs=4, name="seh")
                        for pr in range(2):
                            nc.scalar.activation(
                                attn[:, pr * 1024:(pr + 1) * 1024].rearrange(
                                    "p (a b) -> p a b", a=2),
                                sc_p[pr][:], AF.Exp,
                                bias=negmax[:], scale=1.0,
                                accum_out=seh[:, pr:pr + 1],
                            )
                        nc.vector.reduce_sum(sexp[:, qt:qt + 1], seh[:], axis=AX)
                        attn_q.append(attn)
                        if pending[0] is not None:
                            pending[0][qt]()
                        if qt >= 2:
                            transposes(attn_q, attnT, qt - 2)

                    if qg == 0:
                        fin_extras = qs_extras if qs_extras else tail_extras
                    else:
                        fin_extras = None
                    pending[0] = make_finisher(
                        attn_q, attnT, sexp, h, qg, extras=fin_extras)

                qh = qh_next

            # tail
            ps_ = pending[0]
            ps_[0]()
            ps_[1]()
            ps_[2]()
            ps_[3]()
            final_chunk(1)


_NC_CACHE = None


def _get_nc():
    global _NC_CACHE
    if _NC_CACHE is None:
        nc = bacc.Bacc("TRN2", debug=False, num_devices=NCORES)
        with tile.TileContext(nc) as tc:
            _emit(tc)
        nc.compile()
        _NC_CACHE = nc
    return _NC_CACHE


# --------------------------------------------------------------------------
# Host-side packing
# --------------------------------------------------------------------------

def _fuse(w, blade):
    # einsum('jib,bxy->jixy', w, blade) -> mat[(i,x), (j,y)]
    wb = np.einsum("jib,bxy->jixy", w, blade)
    j, i = w.shape[0], w.shape[1]
    return np.ascontiguousarray(wb.transpose(1, 2, 0, 3)).reshape(i * 16, j * 16)


def _to_kt(m, kparts):
    # [K, F] -> [128, K//128, F] partition-major packing
    k, f = m.shape
    assert k == kparts * 128
    return np.ascontiguousarray(m.reshape(kparts, 128, f).transpose(1, 0, 2))


def _pack_weights(blade, w_enter, w_q, w_k, w_v, w_out, w_final):
    f32 = np.float32
    W_enter = _fuse(w_enter, blade).astype(f32)  # [48, 512]

    Wq = _fuse(w_q, blade)  # [512, 4096], col (j, y), j = d*8 + h
    Wq = Wq.reshape(512, 32, 8, 16)[:, :, :, INNER]   # [c, d, h, yi]
    Wq = Wq.transpose(0, 2, 1, 3).reshape(512, 8, 256)  # [c, h, (d,yi)]
    Wqs = (Wq.reshape(512, 2048) / 16.0).astype(f32)   # fold 1/sqrt(256)

    Wk = _fuse(w_k, blade)  # [512, 512], col (d, y)
    Wks = Wk.reshape(512, 32, 16)[:, :, INNER].reshape(512, 256).astype(f32)

    Wv = _fuse(w_v, blade).astype(f32)       # [512, 512]
    Wo = _fuse(w_out, blade).astype(f32)     # [4096, 512], rows (h, d, x)
    Wf = _fuse(w_final, blade).astype(f32)   # [512, 512]

    maskv = np.zeros(512, f32)
    for d in range(32):
        maskv[d * 16 + INNER] = 1.0 / 32.0
    mask = np.ascontiguousarray(maskv.reshape(4, 128).T)

    return {
        "w_enter": W_enter,
        "w_q": _to_kt(Wqs, 4).astype(np.float16),
        "w_q8": _to_kt(Wqs, 4).astype(ml_dtypes.float8_e5m2),
        "w_k": _to_kt(Wks, 4).astype(np.float16),
        "w_k8": _to_kt(Wks, 4).astype(ml_dtypes.float8_e5m2),
        "w_v": _to_kt(Wv, 4).astype(ml_dtypes.float8_e4m3),
        "w_out": _to_kt(Wo, 32).astype(ml_dtypes.float8_e4m3),
        "w_final": _to_kt(Wf, 4),
        "mask": mask,
        "ident_b": np.eye(128).astype(ml_dtypes.bfloat16),
        "ident_h": np.eye(128).astype(ml_dtypes.float8_e4m3),
    }


def kernel(x, blade, w_enter, w_q, w_k, w_v, w_out, w_final):
    global LAST_RESULTS
    x = np.asarray(x, np.float32)
    shared = _pack_weights(
        np.asarray(blade, np.float32), np.asarray(w_enter, np.float32),
        np.asarray(w_q, np.float32), np.asarray(w_k, np.float32),
        np.asarray(w_v, np.float32), np.asarray(w_out, np.float32),
        np.asarray(w_final, np.float32),
    )

    in_maps = []
    for c in range(NCORES):
        b, half = c // 2, c % 2
        xb = x[b].reshape(S, 48)
        xb = np.roll(xb, -SLAB * half, axis=0)
        m = dict(shared)
        m["xT"] = np.ascontiguousarray(xb.T)
        in_maps.append(m)

    nc = _get_nc()
    res = run_bass_kernel_spmd(
        nc, in_maps, core_ids=list(range(NCORES)), trace=TRACE,
    )
    LAST_RESULTS = res

    out = np.empty((B, S, HID, MV), np.float32)
    for c in range(NCORES):
        b, half = c // 2, c % 2
        outT = res.results[c]["outT"]  # [512, 1024]
        out[b, half * SLAB:(half + 1) * SLAB] = (
            outT.T.reshape(SLAB, HID, MV)
        )
    return out
